# revision 32
# baseline (speedup 1.0000x reference)
"""Trainium2 Bass kernel for ContrastMemoryBankCELoss.

Strategy (8 NeuronCores, SPMD, no collectives) — v2, column-sharded:
  * The 36864 contrast columns (classes 1..18) are sharded across cores:
    core k owns class blocks 2k and 2k+1 (2048 cols each, runs X and Y)
    plus one 512-col quarter of block 16 (k<4) or 17 (k>=4) (run Q).
    All 2048 anchor rows (view-major) are replicated on every core as
    16 groups of 128 partition-rows.
  * Inputs are pre-scaled by sqrt(0.625) and quantized to fp8 e4m3 so a
    single DoubleRow matmul (contraction 256 = 128 partitions x 2
    interleave) yields t = (10/16)*(a.q) in PSUM fp32 at ~216 ns per
    [128,512] tile.
  * Row-wise exp sums are computed by BOTH ScalarE and VectorE in
    parallel:
      - ACT path: activation(Exp, scale=16) with accum_out -> sum exp(u).
      - DVE path: custom fused op EXP16_SQ_ANT computing
        ((1+t)^2+1)^16 = 2^16 * (1+t+t^2/2)^16 ~= 2^16 * exp(16t)
        in one pass from PSUM with accumulate (7 ALU slices + accum).
        Weighted bias of the approximation is ~0.03% (validated 6e-5
        final rel err vs reference).
    A build-time greedy cadence balances the two engines (~47 us each);
    per-(group,run-fragment) partial sums land in per-engine SBUF accum
    tiles and DMA out at the end ([128, 80] fp32 total per core).
  * Everything else is exact fp64 host math: zbs (anchor . class block
    sum), the class-1 diagonal term zd, S = T - B + 2048, and the
    first-order positive-tail formula
      loss_r = -[ zbs - hd*zd - cnt*ln(S) - (B - hd*e^zd)/S ] / cnt,
    which matches the reference to ~2e-7.
"""
import os
import sys

if "/opt/trn_rl_repo" not in sys.path:
    sys.path.insert(0, "/opt/trn_rl_repo")

import numpy as np
import ml_dtypes
from operator import add as _add

FP8 = ml_dtypes.float8_e4m3

A, NVIEW, FEAT, BANK, C = 256, 8, 256, 2048, 19
NROWS = A * NVIEW              # 2048 anchor rows
NBLK = C - 1                   # 18 class blocks
NCOLS = NBLK * BANK            # 36864 contrast columns
NCORES = 8
CPC = NCOLS // NCORES          # 4608 columns per core
NG = NROWS // 128              # 16 row groups
SCALE = float(np.sqrt(0.625))  # joint prescale: t = 0.625 * (a.q) = u/16

# per-group PSUM tile fragments: X = 1536+512, Y = 1536+512, Q = 512
SLOT_SIZES = [1536, 512, 1536, 512, 512]
SLOT_OFF = [0, 1536, 2048, 3584, 4096]

# measured in-kernel per-op engine costs (ns) for cadence balancing
_COST_A = {1536: 1863, 512: 1010}
_COST_D = {1536: 1880, 512: 820}

_PROGRAM = None
LAST_RESULT = None
RUN_KWARGS = {}


# --------------------------------------------------------------------------
# custom DVE op registration
# --------------------------------------------------------------------------
def _register_exp16():
    from concourse import dve_ops as _dve_ops
    from concourse.dve_spec import Spec, Src0, C0, Zero, sq

    def _ref_exp16(in0, in1, s0, s1, imm2):
        t = in0.astype(np.float32)
        b = ((s0 + t) * (s0 + t) + s0).astype(np.float32)
        for _ in range(4):
            b = (b * b).astype(np.float32)
        return b, b.reshape(b.shape[0], -1).sum(-1, keepdims=True).astype(np.float32)

    _s = Src0 + C0
    op = _dve_ops.DveOp(
        "EXP16_SQ_ANT",
        Spec(body=sq(sq(sq(sq(sq(_s) + C0)))), accum=_add, accum_init=Zero,
             reference=_ref_exp16),
        subdim=False,
        uops_sha={"v3": "0f695d0542ee22ff", "v4": "594100af0488a0af"},
    )
    for existing in _dve_ops.OPS:
        if existing.name == op.name:
            return existing
    _dve_ops.OPS.append(op)
    _dve_ops.CUSTOM_DVE_SPECS[op.name] = op.spec
    _dve_ops._SUB_OPCODE_FOR_NAME[op.name] = (
        max(_dve_ops._SUB_OPCODE_FOR_NAME.values()) + 1
    )
    return op


def emission_order(g):
    """Slot emission order per group. Group 0 leads with its three 512
    fragments (whose qt chunks arrive in the first DMA) so the exp engines
    start ~4us earlier while the big chunks stream in."""
    return [1, 3, 4, 0, 2] if g == 0 else [0, 1, 2, 3, 4]


def make_assignment():
    """Greedy engine cadence over measured per-op costs; deterministic on
    host+build. Keyed by (g, slot) in emission order."""
    tA = tD = 0.0
    nA = nD = 0
    out = {}
    for g in range(NG):
        for slot in emission_order(g):
            size = SLOT_SIZES[slot]
            cA, cD = _COST_A[size], _COST_D[size]
            if tA + cA <= tD + cD:
                out[(g, slot)] = ("A", nA)
                tA += cA
                nA += 1
            else:
                out[(g, slot)] = ("D", nD)
                tD += cD
                nD += 1
    return out, nA, nD, tA, tD


def _ensure_ntff_hook():
    """Provide antenv.axon_hooks (NTFF profiling hook) when the image lacks it."""
    import types
    import ctypes
    import contextlib

    try:
        from antenv.axon_hooks import get_axon_ntff_profile_hook  # noqa: F401
        return
    except ImportError:
        pass

    so_path = "/opt/axon/libaxon_pjrt.so"
    if not os.path.exists(so_path):
        return
    try:
        lib = ctypes.CDLL(so_path)
    except OSError:
        return
    if not hasattr(lib, "axon_start_nrt_profile"):
        return
    lib.axon_start_nrt_profile.argtypes = [ctypes.POINTER(ctypes.c_int64),
                                           ctypes.c_size_t]
    lib.axon_start_nrt_profile.restype = ctypes.c_int64
    lib.axon_stop_nrt_profile.argtypes = [ctypes.c_char_p]
    lib.axon_stop_nrt_profile.restype = ctypes.c_int64

    @contextlib.contextmanager
    def _hook(output_dir, device_ids):
        import jax
        jax.devices()
        if device_ids:
            ids = (ctypes.c_int64 * len(device_ids))(*device_ids)
            rc = lib.axon_start_nrt_profile(ids, len(device_ids))
        else:
            rc = lib.axon_start_nrt_profile(None, 0)
        if rc != 0:
            raise RuntimeError(f"axon_start_nrt_profile rc={rc}")
        try:
            yield
        finally:
            n = lib.axon_stop_nrt_profile(str(output_dir).encode())
            print(f"ntff profile: {n} file(s) written to {output_dir}",
                  file=sys.stderr)

    mod = types.ModuleType("antenv.axon_hooks")
    mod.get_axon_ntff_profile_hook = lambda: _hook
    mod.set_axon_ntff_profile_hook = lambda h: None
    sys.modules["antenv.axon_hooks"] = mod


def _build_program():
    from contextlib import ExitStack
    from concourse import bacc, tile, mybir

    exp16 = _register_exp16()
    assign, nA, nD, tA, tD = make_assignment()

    dt = mybir.dt
    fp32 = dt.float32
    bf16 = dt.bfloat16
    f8 = dt.float8e4
    Act = mybir.ActivationFunctionType

    nc = bacc.Bacc("TRN2", target_bir_lowering=False, debug=False,
                   enable_asserts=False, num_devices=NCORES)

    at8a = nc.dram_tensor("at8a", [128, 8, 128], f8, kind="ExternalInput").ap()
    at8b = nc.dram_tensor("at8b", [128, 24, 128], f8, kind="ExternalInput").ap()
    qt8a = nc.dram_tensor("qt8a", [128, 6, 512], f8, kind="ExternalInput").ap()
    qt8b = nc.dram_tensor("qt8b", [128, 6, 512], f8, kind="ExternalInput").ap()
    qt8c = nc.dram_tensor("qt8c", [128, 6, 512], f8, kind="ExternalInput").ap()
    tba = nc.dram_tensor("tba", [128, nA], fp32, kind="ExternalOutput").ap()
    tbd = nc.dram_tensor("tbd", [128, nD], fp32, kind="ExternalOutput").ap()

    with tile.TileContext(nc) as tc, ExitStack() as ctx:
        pers = ctx.enter_context(tc.tile_pool(name="pers", bufs=1))
        pa = ctx.enter_context(tc.tile_pool(name="pa", bufs=2, space="PSUM"))
        pd = ctx.enter_context(tc.tile_pool(name="pd", bufs=2, space="PSUM"))

        atA = pers.tile([128, 8, 128], f8, name="atA", tag="atA")
        atB = pers.tile([128, 24, 128], f8, name="atB", tag="atB")
        qtA = pers.tile([128, 6, 512], f8, name="qtA", tag="qtA")
        qtB = pers.tile([128, 6, 512], f8, name="qtB", tag="qtB")
        qtC = pers.tile([128, 6, 512], f8, name="qtC", tag="qtC")
        scrA = pers.tile([128, 1536], bf16, name="scrA", tag="scrA")
        scrD = pers.tile([128, 1536], bf16, name="scrD", tag="scrD")
        tbA = pers.tile([128, nA], fp32, name="tbA", tag="tbA")
        tbD = pers.tile([128, nD], fp32, name="tbD", tag="tbD")
        warm = pers.tile([128, 8], fp32, name="warm", tag="warm")
        warmo = pers.tile([128, 8], bf16, name="warmo", tag="warmo")

        # parallel triggers on the two fast HWDGE queues (SP + ACT), ordered
        # by first use: atA+qtA (group 0's 512-fragments), then qtB, qtC, atB.
        nc.sync.dma_start(out=atA[:], in_=at8a[:])
        nc.scalar.dma_start(out=qtA[:], in_=qt8a[:])
        nc.sync.dma_start(out=qtB[:], in_=qt8b[:])
        nc.scalar.dma_start(out=qtC[:], in_=qt8c[:])
        nc.sync.dma_start(out=atB[:], in_=at8b[:])

        # early tiny activation so the ~2.7us Exp table load overlaps the
        # input DMAs (output discarded). gpsimd memset: that queue is idle.
        nc.gpsimd.memset(warm[:], 0.0)
        nc.scalar.activation(warmo[:], warm[:], Act.Exp, scale=1.0)

        # PE HAM warm-up: ~4.5us of dummy matmuls during the DMA window so
        # the real matmuls start at the 2.4 GHz clock (K=8/8).
        wsrc = pers.tile([128, 2, 128], f8, name="wsrc", tag="wsrc")
        nc.vector.memset(wsrc[:], 0.0)

        def lhs_of(g):
            return (atA[:, 2 * g:2 * g + 2, :] if g < 4
                    else atB[:, 2 * (g - 4):2 * (g - 4) + 2, :])

        _CHUNK_HOME = {3: (0, 0), 7: (0, 1), 8: (0, 2),
                       0: (1, 0), 1: (1, 1), 2: (1, 2),
                       4: (2, 0), 5: (2, 1), 6: (2, 2)}

        def rhs_of(c):
            t_idx, pos = _CHUNK_HOME[c]
            t = (qtA, qtB, qtC)[t_idx]
            return t[:, 2 * pos:2 * pos + 2, :]

        DR = mybir.MatmulPerfMode.DoubleRow
        twarm = pd.tile([128, 512], fp32, name="twarm", tag="small")
        for _ in range(10):
            nc.tensor.matmul(twarm[:, 0:64], lhsT=wsrc[:],
                             rhs=wsrc[:, :, 0:64], start=True, stop=True,
                             perf_mode=DR)

        for g in range(NG):
            lhs = lhs_of(g)
            for slot in emission_order(g):
                size = SLOT_SIZES[slot]
                pool = pa if size == 1536 else pd
                tag = "big" if size == 1536 else "small"
                t = pool.tile([128, size], fp32, name=f"t{slot}", tag=tag)
                c0 = SLOT_OFF[slot] // 512
                for j in range(size // 512):
                    nc.tensor.matmul(t[:, j * 512:(j + 1) * 512], lhsT=lhs,
                                     rhs=rhs_of(c0 + j), start=True, stop=True,
                                     perf_mode=DR)
                eng, col = assign[(g, slot)]
                if eng == "A":
                    nc.scalar.activation(scrA[:, 0:size], t[:], Act.Exp,
                                         scale=16.0,
                                         accum_out=tbA[:, col:col + 1])
                    if col == nA // 2 - 1:
                        nc.sync.dma_start(out=tba[:, 0:nA // 2],
                                          in_=tbA[:, 0:nA // 2])
                else:
                    nc.vector._custom_dve(exp16, out=scrD[:, 0:size], in0=t[:],
                                          s0=1.0,
                                          accum_out=tbD[:, col:col + 1])
                    if col == nD // 2 - 1:
                        nc.sync.dma_start(out=tbd[:, 0:nD // 2],
                                          in_=tbD[:, 0:nD // 2])

        nc.sync.dma_start(out=tba[:, nA // 2:], in_=tbA[:, nA // 2:])
        nc.sync.dma_start(out=tbd[:, nD // 2:], in_=tbD[:, nD // 2:])

    nc.compile()
    return nc


def _get_program():
    global _PROGRAM
    if _PROGRAM is None:
        _PROGRAM = _build_program()
    return _PROGRAM


def _core_cols(k):
    """Global contrast-column indices owned by core k, in core-local order."""
    x = np.arange(2 * k * BANK, (2 * k + 1) * BANK)
    yy = np.arange((2 * k + 1) * BANK, (2 * k + 2) * BANK)
    qb = 16 if k < 4 else 17
    qq = np.arange(qb * BANK + (k % 4) * 512, qb * BANK + (k % 4) * 512 + 512)
    return np.concatenate([x, yy, qq])


def _stage_inputs(X_anchor, y_anchor, queue):
    X = np.asarray(X_anchor, np.float32)
    Q3 = np.asarray(queue, np.float32)

    AF = X.transpose(1, 0, 2).reshape(NROWS, FEAT)          # view-major rows
    Qm = Q3[1:].reshape(NCOLS, FEAT)                        # classes 1..18

    a8 = (AF * SCALE).astype(FP8)                           # [2048, 256]
    q8 = (Qm * SCALE).astype(FP8)                           # [36864, 256]

    # anchors: at8[p, 2g+i, m] = a8[row=128g+m, feat=128i+p]
    at8 = np.ascontiguousarray(
        a8.reshape(NG, 128, 2, 128).transpose(3, 0, 2, 1).reshape(128, 2 * NG, 128))
    at8a = np.ascontiguousarray(at8[:, 0:8, :])
    at8b = np.ascontiguousarray(at8[:, 8:32, :])

    in_maps = []
    for k in range(NCORES):
        cols = _core_cols(k)
        qk = q8[cols]                                       # [4608, 256]
        # qt8[c, p, i, n] = qk[col=512c+n, feat=128i+p]
        qt8 = np.ascontiguousarray(
            qk.reshape(9, 512, 2, 128).transpose(0, 3, 2, 1))

        def pack(chunks):
            return np.ascontiguousarray(
                np.stack([qt8[c] for c in chunks], 1).reshape(128, 6, 512))

        in_maps.append({"at8a": at8a, "at8b": at8b,
                        "qt8a": pack([3, 7, 8]),
                        "qt8b": pack([0, 1, 2]),
                        "qt8c": pack([4, 5, 6])})
    return in_maps


def kernel(X_anchor, y_anchor, queue):
    global LAST_RESULT
    _ensure_ntff_hook()
    from concourse.bass_utils import run_bass_kernel_spmd

    nc = _get_program()
    in_maps = _stage_inputs(X_anchor, y_anchor, queue)
    res = run_bass_kernel_spmd(nc, in_maps, list(range(NCORES)), **RUN_KWARGS)
    LAST_RESULT = res

    assign, nA, nD, tA, tD = make_assignment()

    X = np.asarray(X_anchor, np.float64)
    y = np.asarray(y_anchor, np.int64)
    Q3 = np.asarray(queue, np.float64)
    AF = X.transpose(1, 0, 2).reshape(NROWS, FEAT)
    Qm = Q3[1:].reshape(NCOLS, FEAT)
    y_rows = np.tile(y, NVIEW)

    # device partial sums -> per-(core, group, slot) fragment sums
    T = np.zeros(NROWS)
    B = np.zeros(NROWS)
    for k in range(NCORES):
        r = res.results[k]
        va = np.asarray(r["tba"], np.float64)
        vd = np.asarray(r["tbd"], np.float64) / 65536.0
        cls_of_slot = [2 * k + 1, 2 * k + 1, 2 * k + 2, 2 * k + 2,
                       17 if k < 4 else 18]
        for g in range(NG):
            rows = slice(g * 128, (g + 1) * 128)
            for slot in range(5):
                eng, col = assign[(g, slot)]
                v = va[:, col] if eng == "A" else vd[:, col]
                T[rows] += v
                m = y_rows[rows] == cls_of_slot[slot]
                B[rows.start:rows.stop][m] += v[m]

    # exact host-side terms
    qbsum = Qm.reshape(NBLK, BANK, FEAT).sum(1)
    ZBS = 10.0 * np.einsum('rf,rf->r', AF, qbsum[y_rows - 1])
    hd = (y_rows == 1).astype(np.float64)
    zd = 10.0 * np.einsum('rf,rf->r', AF, Qm[np.arange(NROWS)]) * hd
    Ed = np.exp(zd) * hd

    S = T - B + float(BANK)
    cnt = float(BANK) - hd
    approx = (ZBS - hd * zd) - cnt * np.log(S) - (B - Ed) / S
    loss = float((-(approx / cnt)).mean())
    return np.float32(loss)
